# revision 29
# baseline (speedup 1.0000x reference)
"""Trainium2 Bass kernel for DerivativeRBF: K(X,X2), grad_K, hess_K.

Math (reference):
  ls = softplus(uls) (D,), var = softplus(uvar)
  Xs = X/ls, X2s = X2/ls
  K[n,m]    = var*exp(-0.25*(|Xs_n|^2 - 2 Xs_n.X2s_m + |X2s_m|^2))      (N,M)
  grad_K    rows d*N+n: -0.5*(X[n,d]-X2[m,d])/ls_d^2 * K[n,m]           (N*D,M)
  kNN[i,j]  = var*exp(-0.25*sqdist(Xs_i,Xs_j))                          (N,N)
  hess_K[a*N+i, b*N+j] = kNN[i,j]*(delta_ab*0.5/ls_a^2
                                   - 0.25*s_a[i,j]*s_b[i,j])            (N*D,N*D)
  with s_d[i,j] = (X[i,d]-X[j,d])/ls_d^2.

Sharding: rows of X split across 8 cores, 64 rows each (SPMD, no
collectives). Each core computes its block-rows of all three outputs;
the host reassembles.

Host prep (tiny, O(N*D)): softplus of the 17 hyperparameters and the
scaled/transposed operand tables below. Device does all the heavy work:
the -0.25*sqdist matmuls + exp, and the D*D grid of N_loc x N hessian
blocks (8.4M f32 per core) plus grad blocks, via fused DVE/GPSIMD ops.

Device-side layout, per core (i0 = 64*core):
  p_d[x] = X[x,d]/ls_d^2;  shat_d[i,j] := p_d[j] - p_d[i] = -s_d[i,j]
  hess block (a,b) = (shat_a * (-0.25*kNN)) * shat_b  (+ delta_ab*c_a*kNN)
  row-pair tile r < 8: partitions 0:64 -> a=2r, 64:128 -> a=2r+1
"""

import sys

if "/opt/trn_rl_repo" not in sys.path:
    sys.path.insert(0, "/opt/trn_rl_repo")

from contextlib import ExitStack

import numpy as np

import concourse.bacc as bacc
import concourse.bass as bass
import concourse.tile as tile
from concourse import mybir

F32 = mybir.dt.float32
AF = mybir.ActivationFunctionType
OP = mybir.AluOpType

N = 512          # rows of X / X2
D = 16           # feature dim
NCORES = 8
NL = N // NCORES  # 64 local rows per core
R = D // 2        # 8 row-pair tiles (two feature dims per 128-partition tile)

# All matmul operands are packed into one [18, MMW] tensor (single DMA ->
# single completion semaphore; the fp32 fused-ldweights Matmult can encode
# only one sync wait, so every matmul may depend on at most one semaphore).
# Column layout of MM18:
#   0:128     L18    rows 0:16 Xs_loc.T dup; row 16 ones; row 17 -0.25*|Xs_loc|^2 dup
#   128:640   RX18   rows 0:16 0.5*Xs.T; row 16 -0.25*|Xs_j|^2; row 17 ones
#   640:1152  RX218  same with X2s
#   1152:1280 sel2   row 0: 1 on cols 0:64; row 1: 1 on cols 64:128
#   1280:1408 onesE  row 0 all ones, row 1 zeros
#   1408:1536 onesO  row 0 zeros, row 1 all ones
#   1536:5632 P2     row 0: p_{2r}[j] r-major; row 1: p_{2r+1}[j]
#   5632:9728 Q2     same from q_d[m] = X2[m,d]/ls_d^2
MMW = 9728
C_L18, C_RX, C_RX2, C_SEL, C_ONE, C_ONO, C_P2, C_Q2 = (
    0, 128, 640, 1152, 1280, 1408, 1536, 5632)
# Per-partition scalar tables (ACT bias / DVE scalar operands) in one
# [128, 32] tensor VT:
#   0:16  PL4N[p, d] = -p_d[i0 + p%64]        (bias for S_all builds)
#   16:24 PL3 col r: p_{2r}[i] upper half, p_{2r+1}[i] lower half
#   24:32 CBU col r: 4*(0.5/ls_{2r}^2) upper half, 0 lower half
#   32:40 CBL col r: 0 upper half, 4*(0.5/ls_{2r+1}^2) lower half
#   (scaled by 4 because the diagonal uses (shat^2 - 4c) * (-0.25*kNN))
#   40:48 PL3N = -PL3 (bias for the shat builds)
#   48:64 PL4N = -PL4 (bias for the GPSIMD-chunk S fixups)
VTW = 64
GP_B = 8  # chunks [D-GP_B, D) go to GPSIMD (minus the diagonal pair)


def _body(ctx, tc, nc, dram):
    MM_d, VT_d, K_d, G_d, H_d = dram

    sing = ctx.enter_context(tc.tile_pool(name="sing", bufs=1))
    ps_b = ctx.enter_context(tc.tile_pool(name="ps_b", bufs=4, space="PSUM"))
    tpool = ctx.enter_context(tc.tile_pool(name="tpool", bufs=3))
    hout = ctx.enter_context(tc.tile_pool(name="hout", bufs=2))
    gout = ctx.enter_context(tc.tile_pool(name="gout", bufs=2))

    MM = sing.tile([D + 2, MMW], F32)
    nc.sync.dma_start(out=MM, in_=MM_d[:, :])
    VT = sing.tile([128, VTW], F32)
    nc.sync.dma_start(out=VT, in_=VT_d[:, :])

    L18 = MM[:, C_L18:C_L18 + 128]
    RX18 = MM[:, C_RX:C_RX + N]
    RX218 = MM[:, C_RX2:C_RX2 + N]
    sel2 = MM[0:2, C_SEL:C_SEL + 128]
    P2 = MM[0:2, C_P2:C_P2 + R * N]
    Q2 = MM[0:2, C_Q2:C_Q2 + R * N]
    PL4 = VT[:, 0:D]
    PL3 = VT[:, D:D + R]
    CBU = VT[:, D + R:D + 2 * R]
    CBL = VT[:, D + 2 * R:D + 3 * R]
    PL3N = VT[:, D + 3 * R:D + 4 * R]
    PL4N = VT[:, D + 4 * R:D + 4 * R + D]

    # ---- S_base: p_b[j] broadcast to all partitions, b-major chunks -----
    # (the -p_b[i] subtraction is fused into the per-chunk STT ops below)
    S_base = sing.tile([128, D * N], F32)
    for b in range(D):
        bcast = bass.AP(
            tensor=MM_d.tensor,
            offset=MM_d.offset + (b % 2) * MMW + C_P2 + (b // 2) * N,
            ap=[[0, 128], [1, N]])
        nc.sync.dma_start(out=S_base[:, b * N:(b + 1) * N], in_=bcast)
    # GPSIMD cannot run STT (Pool engine lacks the ucode), so its chunks
    # get the -p_b[i] subtraction applied in place on ACT, once
    for b in range(D - GP_B, D):
        sl = S_base[:, b * N:(b + 1) * N]
        nc.scalar.activation(out=sl, in_=sl, func=AF.Identity,
                             bias=PL4N[:, b:b + 1])

    # ---- kNN / K: z = -0.25*sqdist via one K=18 matmul each -------------
    zX = ps_b.tile([128, N], F32, tag="pbig")
    nc.tensor.matmul(zX, L18, RX18, start=True, stop=True)
    kNN = sing.tile([128, N], F32)
    nc.scalar.activation(out=kNN, in_=zX, func=AF.Exp)
    kNNq = sing.tile([128, N], F32)  # -0.25 * kNN
    nc.scalar.activation(out=kNNq, in_=kNN, func=AF.Copy, scale=-0.25)

    zK = ps_b.tile([128, N], F32, tag="pbig")
    nc.tensor.matmul(zK, L18, RX218, start=True, stop=True)
    K_dup = sing.tile([128, N], F32)
    nc.scalar.activation(out=K_dup, in_=zK, func=AF.Exp)
    nc.sync.dma_start(out=K_d[:, :], in_=K_dup[0:NL, :])
    K05 = sing.tile([128, N], F32)  # 0.5 * K
    nc.scalar.activation(out=K05, in_=K_dup, func=AF.Copy, scale=0.5)

    # ---- main hessian loop (grad interleaved) ---------------------------
    # H chunk b: (S_base[b] - p_b[i]) * T_r, one STT per chunk. DVE owns
    # chunks 0:D-GP_B (plus the diagonal STTs), GPSIMD owns the rest.
    # H is produced as four [128, 4*N] quarter tiles; quarters 0-1 ship on
    # the sync ring, quarters 2-3 (which wait on GPSIMD) and grad on the
    # ACT ring, so a late quarter can't head-of-line-block the others.
    QW = 4 * N
    for r in range(R):
        # pb = p_{2r}[j] on the upper 64 partitions, p_{2r+1}[j] on the lower
        pb = ps_b.tile([128, N], F32, tag="pbig")
        nc.tensor.matmul(pb, sel2, P2[:, r * N:(r + 1) * N],
                         start=True, stop=True)
        # T_r = shat_a * (-0.25*kNN)   (a = 2r upper half, 2r+1 lower half)
        T_r = tpool.tile([128, N], F32)
        nc.vector.scalar_tensor_tensor(
            out=T_r, in0=pb, scalar=PL3[:, r:r + 1], in1=kNNq,
            op0=OP.subtract, op1=OP.mult)
        # shat_a and shat_a^2 on the scalar engine (for the diagonal blocks)
        shat = tpool.tile([128, N], F32, tag="shat")
        nc.scalar.activation(out=shat, in_=pb, func=AF.Identity,
                             bias=PL3N[:, r:r + 1])
        SQ = tpool.tile([128, N], F32, tag="sq")
        nc.scalar.activation(out=SQ, in_=shat, func=AF.Square)

        qts = [hout.tile([128, QW], F32, tag=f"houtQ{q}", name=f"hq{q}",
                         bufs=3 if q < 2 else 2)
               for q in range(4)]

        def chunk_out(b):
            return qts[b // 4], (b % 4) * N

        # diagonal pair first: diag halves via (shat^2 - 4c) * (-0.25*kNN),
        # off-diagonal halves as 64-partition STTs
        for half, cb in ((0, CBU), (1, CBL)):
            b = 2 * r + half
            t, lo = chunk_out(b)
            dlo, dhi = half * NL, half * NL + NL
            olo, ohi = NL - dlo, 2 * NL - dlo  # the other half
            nc.vector.scalar_tensor_tensor(
                out=t[dlo:dhi, lo:lo + N], in0=SQ[dlo:dhi, :],
                scalar=cb[dlo:dhi, r:r + 1], in1=kNNq[dlo:dhi, :],
                op0=OP.subtract, op1=OP.mult)
            if b < D - GP_B:
                nc.vector.scalar_tensor_tensor(
                    out=t[olo:ohi, lo:lo + N],
                    in0=S_base[olo:ohi, b * N:(b + 1) * N],
                    scalar=PL4[olo:ohi, b:b + 1], in1=T_r[olo:ohi, :],
                    op0=OP.subtract, op1=OP.mult)
            else:
                nc.vector.tensor_mul(t[olo:ohi, lo:lo + N],
                                     T_r[olo:ohi, :],
                                     S_base[olo:ohi, b * N:(b + 1) * N])

        for b in range(D - GP_B):
            if b in (2 * r, 2 * r + 1):
                continue
            t, lo = chunk_out(b)
            nc.vector.scalar_tensor_tensor(
                out=t[:, lo:lo + N], in0=S_base[:, b * N:(b + 1) * N],
                scalar=PL4[:, b:b + 1], in1=T_r,
                op0=OP.subtract, op1=OP.mult)

        def tb(k):
            return bass.AP(tensor=T_r.tensor, offset=T_r.offset,
                           ap=[T_r.ap[0], [0, k], T_r.ap[1]])

        run = []
        for b in [x for x in range(D - GP_B, D)
                  if x not in (2 * r, 2 * r + 1)] + [None]:
            if b is not None and (not run or b == run[-1] + 1):
                run.append(b)
                continue
            if run:
                # runs stay within one quarter tile (chunks 10..15 span
                # quarters 2 and 3); split at the quarter boundary
                segs = {}
                for x in run:
                    segs.setdefault(x // 4, []).append(x)
                for q, xs in segs.items():
                    t, lo = chunk_out(xs[0])
                    num = len(xs)
                    nc.gpsimd.tensor_mul(
                        t[:, lo:lo + num * N].rearrange(
                            "p (b j) -> p b j", b=num),
                        tb(num),
                        S_base[:, xs[0] * N:(xs[0] + num) * N].rearrange(
                            "p (b j) -> p b j", b=num))
            run = [b]

        for q in range(4):
            dma_eng = nc.sync if q < 2 else nc.scalar
            dma_eng.dma_start(
                out=H_d[r * 128:(r + 1) * 128, q * QW:(q + 1) * QW],
                in_=qts[q])

        # one grad tile per iteration (ACT-ring DMA, emitted before the
        # GP-gated quarters of the next iteration)
        qb = ps_b.tile([128, N], F32, tag="pbig")
        nc.tensor.matmul(qb, sel2, Q2[:, r * N:(r + 1) * N],
                         start=True, stop=True)
        G_t = gout.tile([128, N], F32)
        nc.vector.scalar_tensor_tensor(
            out=G_t, in0=qb, scalar=PL3[:, r:r + 1], in1=K05,
            op0=OP.subtract, op1=OP.mult)
        nc.scalar.dma_start(out=G_d[r * 128:(r + 1) * 128, :], in_=G_t)


def build_nc():
    nc = bacc.Bacc()
    MM_d = nc.dram_tensor("MM18", [D + 2, MMW], F32,
                          kind="ExternalInput").ap()
    VT_d = nc.dram_tensor("VT", [128, VTW], F32, kind="ExternalInput").ap()
    K_d = nc.dram_tensor("Kout", [NL, N], F32, kind="ExternalOutput").ap()
    G_d = nc.dram_tensor("Gout", [NL * D, N], F32, kind="ExternalOutput").ap()
    H_d = nc.dram_tensor("Hout", [NL * D, N * D], F32,
                         kind="ExternalOutput").ap()
    with tile.TileContext(nc) as tc:
        with ExitStack() as ctx:
            _body(ctx, tc, nc, (MM_d, VT_d, K_d, G_d, H_d))
    # Bacc lowering: splits multi-sem waits into EventSemaphore instructions
    # (walrus allows at most one sync wait per engine instruction on TRN2),
    # moves matmul waits to ldweights, allocates registers.
    nc.compile()
    return nc


_CACHE = {}


def get_nc():
    if "nc" not in _CACHE:
        _CACHE["nc"] = build_nc()
    return _CACHE["nc"]


def make_in_maps(X, X2, uls, uvar):
    """Host prep: softplus the 17 hyperparameters and pack the small operand
    tables (O(N*D) f64 math) into MM18/VT; shard local-row tables per core."""
    X = np.asarray(X, np.float64)
    X2 = np.asarray(X2, np.float64)
    uls = np.asarray(uls, np.float64)
    uvar = np.asarray(uvar, np.float64)

    ls = np.logaddexp(0.0, uls)            # softplus
    var = np.logaddexp(0.0, uvar)[0]
    linv2 = 1.0 / (ls * ls)

    Xs = X / ls
    X2s = X2 / ls
    P = X * linv2                          # (N, D): p_d[x]
    Q = X2 * linv2                         # (N, D): q_d[m]
    nX = -0.25 * np.sum(Xs * Xs, axis=1)   # (N,)
    nX2 = -0.25 * np.sum(X2s * X2s, axis=1)

    mm = np.zeros((D + 2, MMW))
    mm[0:D, C_RX:C_RX + N] = 0.5 * Xs.T
    mm[D, C_RX:C_RX + N] = nX
    mm[D + 1, C_RX:C_RX + N] = 1.0
    mm[0:D, C_RX2:C_RX2 + N] = 0.5 * X2s.T
    mm[D, C_RX2:C_RX2 + N] = nX2
    mm[D + 1, C_RX2:C_RX2 + N] = 1.0
    mm[0, C_SEL:C_SEL + 64] = 1.0
    mm[1, C_SEL + 64:C_SEL + 128] = 1.0
    mm[0, C_ONE:C_ONE + 128] = 1.0
    mm[1, C_ONO:C_ONO + 128] = 1.0
    mm[0:2, C_P2:C_P2 + R * N] = \
        P.T.reshape(R, 2, N).transpose(1, 0, 2).reshape(2, R * N)
    mm[0:2, C_Q2:C_Q2 + R * N] = \
        Q.T.reshape(R, 2, N).transpose(1, 0, 2).reshape(2, R * N)

    vt = np.zeros((128, VTW))
    vt[0:64, D + R:D + 2 * R] = (2.0 * linv2)[0::2][None, :]
    vt[64:128, D + 2 * R:D + 3 * R] = (2.0 * linv2)[1::2][None, :]

    maps = []
    for c in range(NCORES):
        rows = slice(c * NL, (c + 1) * NL)
        Xl = Xs[rows]                       # (64, D)
        Pl = P[rows]                        # (64, D)
        nl = -0.25 * np.sum(Xl * Xl, axis=1)
        mmc = mm.copy()
        mmc[0:D, C_L18:C_L18 + 64] = Xl.T
        mmc[0:D, C_L18 + 64:C_L18 + 128] = Xl.T
        mmc[D, C_L18:C_L18 + 128] = 1.0
        mmc[D + 1, C_L18:C_L18 + 64] = nl + np.log(var)
        mmc[D + 1, C_L18 + 64:C_L18 + 128] = nl + np.log(var)
        vtc = vt.copy()
        vtc[0:64, 0:D] = Pl
        vtc[64:128, 0:D] = Pl
        vtc[0:64, D:D + R] = Pl[:, 0::2]
        vtc[64:128, D:D + R] = Pl[:, 1::2]
        vtc[0:64, D + 3 * R:D + 4 * R] = -Pl[:, 0::2]
        vtc[64:128, D + 3 * R:D + 4 * R] = -Pl[:, 1::2]
        vtc[0:64, D + 4 * R:D + 4 * R + D] = -Pl
        vtc[64:128, D + 4 * R:D + 4 * R + D] = -Pl
        maps.append({
            "MM18": np.ascontiguousarray(mmc, dtype=np.float32),
            "VT": np.ascontiguousarray(vtc, dtype=np.float32),
        })
    return maps


def assemble(results):
    K = np.empty((N, N), np.float32)
    G = np.empty((N * D, N), np.float32)
    H = np.empty((N * D, N * D), np.float32)
    Gr = G.reshape(D, NCORES, NL, N)
    Hr = H.reshape(D, NCORES, NL, N * D)
    for c, res in enumerate(results):
        K[c * NL:(c + 1) * NL] = res["Kout"]
        Gr[:, c] = res["Gout"].reshape(D, NL, N)
        Hr[:, c] = res["Hout"].reshape(D, NL, N * D)
    return K, G, H


def run(X, X2, uls, uvar, trace=False, **kw):
    from concourse.bass_utils import run_bass_kernel_spmd

    nc = get_nc()
    in_maps = make_in_maps(X, X2, uls, uvar)
    out = run_bass_kernel_spmd(nc, in_maps, core_ids=list(range(NCORES)),
                               trace=trace, **kw)
    return assemble(out.results), out


def kernel(X, X2, uls, uvar):
    (K, G, H), _ = run(X, X2, uls, uvar)
    return K, G, H


# revision 30
# speedup vs baseline: 1.0585x; 1.0585x over previous
"""Trainium2 Bass kernel for DerivativeRBF: K(X,X2), grad_K, hess_K.

Math (reference):
  ls = softplus(uls) (D,), var = softplus(uvar)
  Xs = X/ls, X2s = X2/ls
  K[n,m]    = var*exp(-0.25*(|Xs_n|^2 - 2 Xs_n.X2s_m + |X2s_m|^2))      (N,M)
  grad_K    rows d*N+n: -0.5*(X[n,d]-X2[m,d])/ls_d^2 * K[n,m]           (N*D,M)
  kNN[i,j]  = var*exp(-0.25*sqdist(Xs_i,Xs_j))                          (N,N)
  hess_K[a*N+i, b*N+j] = kNN[i,j]*(delta_ab*0.5/ls_a^2
                                   - 0.25*s_a[i,j]*s_b[i,j])            (N*D,N*D)
  with s_d[i,j] = (X[i,d]-X[j,d])/ls_d^2.

Sharding: rows of X split across 8 cores, 64 rows each (SPMD, no
collectives). Each core computes its block-rows of all three outputs;
the host reassembles.

Host prep (tiny, O(N*D)): softplus of the 17 hyperparameters and the
scaled/transposed operand tables below. Device does all the heavy work:
the -0.25*sqdist matmuls + exp, and the D*D grid of N_loc x N hessian
blocks (8.4M f32 per core) plus grad blocks, via fused DVE/GPSIMD ops.

Device-side layout, per core (i0 = 64*core):
  p_d[x] = X[x,d]/ls_d^2;  shat_d[i,j] := p_d[j] - p_d[i] = -s_d[i,j]
  hess block (a,b) = (shat_a * (-0.25*kNN)) * shat_b  (+ delta_ab*c_a*kNN)
  row-pair tile r < 8: partitions 0:64 -> a=2r, 64:128 -> a=2r+1
"""

import sys

if "/opt/trn_rl_repo" not in sys.path:
    sys.path.insert(0, "/opt/trn_rl_repo")

from contextlib import ExitStack

import numpy as np

import concourse.bacc as bacc
import concourse.bass as bass
import concourse.tile as tile
from concourse import mybir

F32 = mybir.dt.float32
AF = mybir.ActivationFunctionType
OP = mybir.AluOpType

N = 512          # rows of X / X2
D = 16           # feature dim
NCORES = 8
NL = N // NCORES  # 64 local rows per core
R = D // 2        # 8 row-pair tiles (two feature dims per 128-partition tile)

# All matmul operands are packed into one [18, MMW] tensor (single DMA ->
# single completion semaphore; the fp32 fused-ldweights Matmult can encode
# only one sync wait, so every matmul may depend on at most one semaphore).
# Column layout of MM18:
#   0:128     L18    rows 0:16 Xs_loc.T dup; row 16 ones; row 17 -0.25*|Xs_loc|^2 dup
#   128:640   RX18   rows 0:16 0.5*Xs.T; row 16 -0.25*|Xs_j|^2; row 17 ones
#   640:1152  RX218  same with X2s
#   1152:1280 sel2   row 0: 1 on cols 0:64; row 1: 1 on cols 64:128
#   1280:1408 onesE  row 0 all ones, row 1 zeros
#   1408:1536 onesO  row 0 zeros, row 1 all ones
#   1536:5632 P2     row 0: p_{2r}[j] r-major; row 1: p_{2r+1}[j]
#   5632:9728 Q2     same from q_d[m] = X2[m,d]/ls_d^2
MMW = 9728
C_L18, C_RX, C_RX2, C_SEL, C_ONE, C_ONO, C_P2, C_Q2 = (
    0, 128, 640, 1152, 1280, 1408, 1536, 5632)
# Per-partition scalar tables (ACT bias / DVE scalar operands) in one
# [128, 32] tensor VT:
#   0:16  PL4N[p, d] = -p_d[i0 + p%64]        (bias for S_all builds)
#   16:24 PL3 col r: p_{2r}[i] upper half, p_{2r+1}[i] lower half
#   24:32 CBU col r: 4*(0.5/ls_{2r}^2) upper half, 0 lower half
#   32:40 CBL col r: 0 upper half, 4*(0.5/ls_{2r+1}^2) lower half
#   (scaled by 4 because the diagonal uses (shat^2 - 4c) * (-0.25*kNN))
#   40:48 PL3N = -PL3 (bias for the shat builds)
#   48:64 PL4N = -PL4 (bias for the GPSIMD-chunk S fixups)
VTW = 64
GP_B = 6  # chunks [D-GP_B, D) go to GPSIMD (minus the diagonal pair)


def _body(ctx, tc, nc, dram):
    MM_d, VT_d, K_d, G_d, H_d = dram

    sing = ctx.enter_context(tc.tile_pool(name="sing", bufs=1))
    ps_b = ctx.enter_context(tc.tile_pool(name="ps_b", bufs=4, space="PSUM"))
    tpool = ctx.enter_context(tc.tile_pool(name="tpool", bufs=3))
    hout = ctx.enter_context(tc.tile_pool(name="hout", bufs=2))
    gout = ctx.enter_context(tc.tile_pool(name="gout", bufs=2))

    MM = sing.tile([D + 2, MMW], F32)
    nc.sync.dma_start(out=MM, in_=MM_d[:, :])
    VT = sing.tile([128, VTW], F32)
    nc.sync.dma_start(out=VT, in_=VT_d[:, :])

    L18 = MM[:, C_L18:C_L18 + 128]
    RX18 = MM[:, C_RX:C_RX + N]
    RX218 = MM[:, C_RX2:C_RX2 + N]
    sel2 = MM[0:2, C_SEL:C_SEL + 128]
    P2 = MM[0:2, C_P2:C_P2 + R * N]
    Q2 = MM[0:2, C_Q2:C_Q2 + R * N]
    PL4 = VT[:, 0:D]
    PL3 = VT[:, D:D + R]
    CBU = VT[:, D + R:D + 2 * R]
    CBL = VT[:, D + 2 * R:D + 3 * R]
    PL3N = VT[:, D + 3 * R:D + 4 * R]
    PL4N = VT[:, D + 4 * R:D + 4 * R + D]

    # ---- S_base: p_b[j] broadcast to all partitions, b-major chunks -----
    # (the -p_b[i] subtraction is fused into the per-chunk STT ops below)
    S_base = sing.tile([128, D * N], F32)
    for b in range(D):
        bcast = bass.AP(
            tensor=MM_d.tensor,
            offset=MM_d.offset + (b % 2) * MMW + C_P2 + (b // 2) * N,
            ap=[[0, 128], [1, N]])
        nc.sync.dma_start(out=S_base[:, b * N:(b + 1) * N], in_=bcast)
    # GPSIMD cannot run STT (Pool engine lacks the ucode), so its chunks
    # get the -p_b[i] subtraction applied in place on ACT, once
    for b in range(D - GP_B, D):
        sl = S_base[:, b * N:(b + 1) * N]
        nc.scalar.activation(out=sl, in_=sl, func=AF.Identity,
                             bias=PL4N[:, b:b + 1])

    # ---- kNN / K: z = -0.25*sqdist via one K=18 matmul each -------------
    zX = ps_b.tile([128, N], F32, tag="pbig")
    nc.tensor.matmul(zX, L18, RX18, start=True, stop=True)
    kNN = sing.tile([128, N], F32)
    nc.scalar.activation(out=kNN, in_=zX, func=AF.Exp)
    kNNq = sing.tile([128, N], F32)  # -0.25 * kNN
    nc.scalar.activation(out=kNNq, in_=kNN, func=AF.Copy, scale=-0.25)

    zK = ps_b.tile([128, N], F32, tag="pbig")
    nc.tensor.matmul(zK, L18, RX218, start=True, stop=True)
    K_dup = sing.tile([128, N], F32)
    nc.scalar.activation(out=K_dup, in_=zK, func=AF.Exp)
    nc.sync.dma_start(out=K_d[:, :], in_=K_dup[0:NL, :])
    K05 = sing.tile([128, N], F32)  # 0.5 * K
    nc.scalar.activation(out=K05, in_=K_dup, func=AF.Copy, scale=0.5)

    # ---- main hessian loop (grad interleaved) ---------------------------
    # H chunk b: (S_base[b] - p_b[i]) * T_r, one STT per chunk. DVE owns
    # chunks 0:D-GP_B (plus the diagonal STTs), GPSIMD owns the rest.
    # H is produced as four [128, 4*N] quarter tiles; quarters 0-1 ship on
    # the sync ring, quarters 2-3 (which wait on GPSIMD) and grad on the
    # ACT ring, so a late quarter can't head-of-line-block the others.
    QW = 4 * N
    for r in range(R):
        # pb = p_{2r}[j] on the upper 64 partitions, p_{2r+1}[j] on the lower
        pb = ps_b.tile([128, N], F32, tag="pbig")
        nc.tensor.matmul(pb, sel2, P2[:, r * N:(r + 1) * N],
                         start=True, stop=True)
        # T_r = shat_a * (-0.25*kNN)   (a = 2r upper half, 2r+1 lower half)
        T_r = tpool.tile([128, N], F32)
        nc.vector.scalar_tensor_tensor(
            out=T_r, in0=pb, scalar=PL3[:, r:r + 1], in1=kNNq,
            op0=OP.subtract, op1=OP.mult)
        # shat_a and shat_a^2 on the scalar engine (for the diagonal blocks)
        shat = tpool.tile([128, N], F32, tag="shat")
        nc.scalar.activation(out=shat, in_=pb, func=AF.Identity,
                             bias=PL3N[:, r:r + 1])
        SQ = tpool.tile([128, N], F32, tag="sq")
        nc.scalar.activation(out=SQ, in_=shat, func=AF.Square)

        qts = [hout.tile([128, QW], F32, tag=f"houtQ{q}", name=f"hq{q}",
                         bufs=3 if q < 2 else 2)
               for q in range(4)]

        def chunk_out(b):
            return qts[b // 4], (b % 4) * N

        # diagonal pair first: diag halves via (shat^2 - 4c) * (-0.25*kNN),
        # off-diagonal halves as 64-partition STTs
        for half, cb in ((0, CBU), (1, CBL)):
            b = 2 * r + half
            t, lo = chunk_out(b)
            dlo, dhi = half * NL, half * NL + NL
            olo, ohi = NL - dlo, 2 * NL - dlo  # the other half
            nc.vector.scalar_tensor_tensor(
                out=t[dlo:dhi, lo:lo + N], in0=SQ[dlo:dhi, :],
                scalar=cb[dlo:dhi, r:r + 1], in1=kNNq[dlo:dhi, :],
                op0=OP.subtract, op1=OP.mult)
            if b < D - GP_B:
                nc.vector.scalar_tensor_tensor(
                    out=t[olo:ohi, lo:lo + N],
                    in0=S_base[olo:ohi, b * N:(b + 1) * N],
                    scalar=PL4[olo:ohi, b:b + 1], in1=T_r[olo:ohi, :],
                    op0=OP.subtract, op1=OP.mult)
            else:
                nc.vector.tensor_mul(t[olo:ohi, lo:lo + N],
                                     T_r[olo:ohi, :],
                                     S_base[olo:ohi, b * N:(b + 1) * N])

        for b in range(D - GP_B):
            if b in (2 * r, 2 * r + 1):
                continue
            t, lo = chunk_out(b)
            nc.vector.scalar_tensor_tensor(
                out=t[:, lo:lo + N], in0=S_base[:, b * N:(b + 1) * N],
                scalar=PL4[:, b:b + 1], in1=T_r,
                op0=OP.subtract, op1=OP.mult)

        def tb(k):
            return bass.AP(tensor=T_r.tensor, offset=T_r.offset,
                           ap=[T_r.ap[0], [0, k], T_r.ap[1]])

        run = []
        for b in [x for x in range(D - GP_B, D)
                  if x not in (2 * r, 2 * r + 1)] + [None]:
            if b is not None and (not run or b == run[-1] + 1):
                run.append(b)
                continue
            if run:
                # runs stay within one quarter tile (chunks 10..15 span
                # quarters 2 and 3); split at the quarter boundary
                segs = {}
                for x in run:
                    segs.setdefault(x // 4, []).append(x)
                for q, xs in segs.items():
                    t, lo = chunk_out(xs[0])
                    num = len(xs)
                    nc.gpsimd.tensor_mul(
                        t[:, lo:lo + num * N].rearrange(
                            "p (b j) -> p b j", b=num),
                        tb(num),
                        S_base[:, xs[0] * N:(xs[0] + num) * N].rearrange(
                            "p (b j) -> p b j", b=num))
            run = [b]

        for q in range(4):
            dma_eng = nc.sync if q < 2 else nc.scalar
            dma_eng.dma_start(
                out=H_d[r * 128:(r + 1) * 128, q * QW:(q + 1) * QW],
                in_=qts[q])

        # one grad tile per iteration (ACT-ring DMA, emitted before the
        # GP-gated quarters of the next iteration)
        qb = ps_b.tile([128, N], F32, tag="pbig")
        nc.tensor.matmul(qb, sel2, Q2[:, r * N:(r + 1) * N],
                         start=True, stop=True)
        G_t = gout.tile([128, N], F32)
        nc.vector.scalar_tensor_tensor(
            out=G_t, in0=qb, scalar=PL3[:, r:r + 1], in1=K05,
            op0=OP.subtract, op1=OP.mult)
        nc.scalar.dma_start(out=G_d[r * 128:(r + 1) * 128, :], in_=G_t)


def build_nc():
    nc = bacc.Bacc()
    MM_d = nc.dram_tensor("MM18", [D + 2, MMW], F32,
                          kind="ExternalInput").ap()
    VT_d = nc.dram_tensor("VT", [128, VTW], F32, kind="ExternalInput").ap()
    K_d = nc.dram_tensor("Kout", [NL, N], F32, kind="ExternalOutput").ap()
    G_d = nc.dram_tensor("Gout", [NL * D, N], F32, kind="ExternalOutput").ap()
    H_d = nc.dram_tensor("Hout", [NL * D, N * D], F32,
                         kind="ExternalOutput").ap()
    with tile.TileContext(nc) as tc:
        with ExitStack() as ctx:
            _body(ctx, tc, nc, (MM_d, VT_d, K_d, G_d, H_d))
    # Bacc lowering: splits multi-sem waits into EventSemaphore instructions
    # (walrus allows at most one sync wait per engine instruction on TRN2),
    # moves matmul waits to ldweights, allocates registers.
    nc.compile()
    return nc


_CACHE = {}


def get_nc():
    if "nc" not in _CACHE:
        _CACHE["nc"] = build_nc()
    return _CACHE["nc"]


def make_in_maps(X, X2, uls, uvar):
    """Host prep: softplus the 17 hyperparameters and pack the small operand
    tables (O(N*D) f64 math) into MM18/VT; shard local-row tables per core."""
    X = np.asarray(X, np.float64)
    X2 = np.asarray(X2, np.float64)
    uls = np.asarray(uls, np.float64)
    uvar = np.asarray(uvar, np.float64)

    ls = np.logaddexp(0.0, uls)            # softplus
    var = np.logaddexp(0.0, uvar)[0]
    linv2 = 1.0 / (ls * ls)

    Xs = X / ls
    X2s = X2 / ls
    P = X * linv2                          # (N, D): p_d[x]
    Q = X2 * linv2                         # (N, D): q_d[m]
    nX = -0.25 * np.sum(Xs * Xs, axis=1)   # (N,)
    nX2 = -0.25 * np.sum(X2s * X2s, axis=1)

    mm = np.zeros((D + 2, MMW))
    mm[0:D, C_RX:C_RX + N] = 0.5 * Xs.T
    mm[D, C_RX:C_RX + N] = nX
    mm[D + 1, C_RX:C_RX + N] = 1.0
    mm[0:D, C_RX2:C_RX2 + N] = 0.5 * X2s.T
    mm[D, C_RX2:C_RX2 + N] = nX2
    mm[D + 1, C_RX2:C_RX2 + N] = 1.0
    mm[0, C_SEL:C_SEL + 64] = 1.0
    mm[1, C_SEL + 64:C_SEL + 128] = 1.0
    mm[0, C_ONE:C_ONE + 128] = 1.0
    mm[1, C_ONO:C_ONO + 128] = 1.0
    mm[0:2, C_P2:C_P2 + R * N] = \
        P.T.reshape(R, 2, N).transpose(1, 0, 2).reshape(2, R * N)
    mm[0:2, C_Q2:C_Q2 + R * N] = \
        Q.T.reshape(R, 2, N).transpose(1, 0, 2).reshape(2, R * N)

    vt = np.zeros((128, VTW))
    vt[0:64, D + R:D + 2 * R] = (2.0 * linv2)[0::2][None, :]
    vt[64:128, D + 2 * R:D + 3 * R] = (2.0 * linv2)[1::2][None, :]

    maps = []
    for c in range(NCORES):
        rows = slice(c * NL, (c + 1) * NL)
        Xl = Xs[rows]                       # (64, D)
        Pl = P[rows]                        # (64, D)
        nl = -0.25 * np.sum(Xl * Xl, axis=1)
        mmc = mm.copy()
        mmc[0:D, C_L18:C_L18 + 64] = Xl.T
        mmc[0:D, C_L18 + 64:C_L18 + 128] = Xl.T
        mmc[D, C_L18:C_L18 + 128] = 1.0
        mmc[D + 1, C_L18:C_L18 + 64] = nl + np.log(var)
        mmc[D + 1, C_L18 + 64:C_L18 + 128] = nl + np.log(var)
        vtc = vt.copy()
        vtc[0:64, 0:D] = Pl
        vtc[64:128, 0:D] = Pl
        vtc[0:64, D:D + R] = Pl[:, 0::2]
        vtc[64:128, D:D + R] = Pl[:, 1::2]
        vtc[0:64, D + 3 * R:D + 4 * R] = -Pl[:, 0::2]
        vtc[64:128, D + 3 * R:D + 4 * R] = -Pl[:, 1::2]
        vtc[0:64, D + 4 * R:D + 4 * R + D] = -Pl
        vtc[64:128, D + 4 * R:D + 4 * R + D] = -Pl
        maps.append({
            "MM18": np.ascontiguousarray(mmc, dtype=np.float32),
            "VT": np.ascontiguousarray(vtc, dtype=np.float32),
        })
    return maps


def assemble(results):
    K = np.empty((N, N), np.float32)
    G = np.empty((N * D, N), np.float32)
    H = np.empty((N * D, N * D), np.float32)
    Gr = G.reshape(D, NCORES, NL, N)
    Hr = H.reshape(D, NCORES, NL, N * D)
    for c, res in enumerate(results):
        K[c * NL:(c + 1) * NL] = res["Kout"]
        Gr[:, c] = res["Gout"].reshape(D, NL, N)
        Hr[:, c] = res["Hout"].reshape(D, NL, N * D)
    return K, G, H


def run(X, X2, uls, uvar, trace=False, **kw):
    from concourse.bass_utils import run_bass_kernel_spmd

    nc = get_nc()
    in_maps = make_in_maps(X, X2, uls, uvar)
    out = run_bass_kernel_spmd(nc, in_maps, core_ids=list(range(NCORES)),
                               trace=trace, **kw)
    return assemble(out.results), out


def kernel(X, X2, uls, uvar):
    (K, G, H), _ = run(X, X2, uls, uvar)
    return K, G, H
